# revision 1
# baseline (speedup 1.0000x reference)
"""AffinityPropagate prediction kernel for Trainium2 (8 NeuronCores).

Data-parallel over batch B=8: each core owns one image [480, 640].

Layout per core: 120 partitions x 4 image rows. Rows padded to 642 cols
(zero col at each end). State kept in fp32 (single in-place buffer, no
halo slots). Each iteration builds scaled fp16 copies fb (and fbs = fb
shifted one element left, serving the 2B-misaligned dc==0 taps) --
double-buffered -- with 2 halo row slots refreshed by SBUF->SBUF DMA.

Per iteration (matches reference math):
  f_new = w_center*f + sum_{8 taps} w_t * shift_t(f)
Center product + final add run in fp32 (dominates accuracy); the 8
other products and their pairwise-tree sum run in fp16 at DVE 2x mode,
on values scaled by 2^-iter to stay inside fp16 range (|w_t| <= 1
guarantees no overflow; the unscale folds into the final fused op).
Ops are emitted in 2 row-groups so ACT copies + halo DMAs overlap DVE.
"""

import numpy as np
from contextlib import ExitStack

import concourse.bacc as bacc
import concourse.mybir as mybir
import concourse.tile as tile
from concourse import bass_utils
from concourse.bass_interp import get_hw_module

B, CH, H, W = 8, 8, 480, 640
P = 120            # partitions used (each holds ROWS_P rows)
ROWS_P = H // P    # 4
SLOTS = ROWS_P + 2  # fb/fbs: + top/bottom halo row slots
WPAD = W + 2       # col-padded row: [0, img cols at 1..640, 0]
PX = ROWS_P * W    # 2560 compact px per partition

F32 = mybir.dt.float32
FP16 = mybir.dt.float16
AF = mybir.ActivationFunctionType
OP = mybir.AluOpType

# 8 non-center taps in reference slab order: (dr, dc); w8b slab i = TAPS8[i]
TAPS8 = [(t // 3 - 1, t % 3 - 1) for t in range(9) if t != 4]


def _build(times: int):
    nc = bacc.Bacc("TRN2", debug=False, dynamic_dma_scratch_size=2048)
    aff_d = nc.dram_tensor("affinity", [CH, H * W], F32, kind="ExternalInput")
    feat_d = nc.dram_tensor("feature", [H, W], F32, kind="ExternalInput")
    out_d = nc.dram_tensor("out", [H, W], F32, kind="ExternalOutput")

    with tile.TileContext(nc) as tc, ExitStack() as ctx:
        pool = ctx.enter_context(tc.tile_pool(name="main", bufs=1))

        w8b = pool.tile([P, 8, PX], FP16)          # fp16 tap weights
        w4f = pool.tile([P, PX], F32)              # fp32 center weight
        f32s = pool.tile([P, ROWS_P * WPAD], F32)  # fp32 state (in-place)
        fbp = [pool.tile([P, SLOTS * WPAD], FP16, name=f"fb{i}")
               for i in range(2)]
        fbsp = [pool.tile([P, SLOTS * WPAD], FP16, name=f"fbs{i}")
                for i in range(2)]
        # fp16 scratch for products/tree, per row-group
        pr = [[pool.tile([P, 2 * W], FP16, name=f"pr{g}_{i}") for i in range(4)]
              for g in range(2)]
        sums = pool.tile([P, PX], F32)
        suma = pool.tile([P, PX], F32)
        rec = pool.tile([P, PX], F32)
        rec16 = pool.tile([P, PX], FP16)
        # scr shares slots with the per-iteration c32 ring
        scr = pool.tile([P, PX], F32, tag="c32", bufs=2)

        def f3(t):
            return t[:, :].rearrange("p (s w) -> p s w", w=WPAD)

        fs3 = f3(f32s)
        fbv = [f3(t) for t in fbp]
        fbsv = [f3(t) for t in fbsp]

        # ---- zero-init padded buffers ----
        nc.vector.memset(f32s[:, :], 0.0)
        for t in fbp + fbsp:
            nc.vector.memset(t[:, :], 0.0)

        # ---- load feature ----
        feat_v = feat_d[:, :].rearrange("(p r) w -> p r w", r=ROWS_P)
        nc.sync.dma_start(fs3[:, :, 1:1 + W], feat_v)

        # ---- load affinity (channel at a time) + weight prep ----
        aff_v = aff_d[:, :].rearrange("c (p x) -> c p x", x=PX)
        for c in range(CH):
            st = pool.tile([P, PX], F32, name=f"stage{c}", tag="stg", bufs=2)
            # alternate HWDGE rings so channel loads pipeline
            (nc.sync if c % 2 == 0 else nc.scalar).dma_start(
                st[:, :], aff_v[c])
            # fp16 copy of channel into weight slab (normalized later)
            nc.scalar.activation(w8b[:, c, :], st[:, :], AF.Copy)
            if c == 0:
                nc.scalar.activation(sums[:, :], st[:, :], AF.Abs)
                nc.vector.tensor_copy(suma[:, :], st[:, :])
            else:
                absdst = rec if c % 2 else w4f
                nc.scalar.activation(absdst[:, :], st[:, :], AF.Abs)
                nc.vector.tensor_add(sums[:, :], sums[:, :], absdst[:, :])
                nc.vector.tensor_add(suma[:, :], suma[:, :], st[:, :])
        nc.vector.reciprocal_approx_accurate(rec[:, :], sums[:, :], scr[:, :])
        # fp16 copy of rec so slab normalization runs in DVE 2x mode
        nc.scalar.activation(rec16[:, :], rec[:, :], AF.Copy)
        for i in range(8):
            nc.vector.tensor_mul(w8b[:, i, :], w8b[:, i, :], rec16[:, :])
        # center = 1 - suma * rec  (fp32)
        nc.vector.scalar_tensor_tensor(w4f[:, :], suma[:, :], -1.0, rec[:, :],
                                       OP.mult, OP.mult)
        nc.vector.tensor_scalar_add(w4f[:, :], w4f[:, :], 1.0)

        def make_f16_copies(dfb, dfbs, scale, g):
            # fp16 scaled copies of f32 state rows of group g
            r0 = 2 * g
            nc.scalar.activation(dfb[:, 1 + r0:3 + r0, :],
                                 fs3[:, r0:r0 + 2, :], AF.Copy, scale=scale)
            nc.scalar.activation(dfbs[:, 1 + r0:3 + r0, 0:W + 1],
                                 fs3[:, r0:r0 + 2, 1:WPAD], AF.Copy,
                                 scale=scale)

        def halo_dmas(dfb, dfbs):
            # two HWDGE rings (sync + scalar) to avoid head-of-line blocking
            nc.sync.dma_start(dfb[1:P, 0, :], dfb[0:P - 1, ROWS_P, :])
            nc.scalar.dma_start(dfb[0:P - 1, SLOTS - 1, :], dfb[1:P, 1, :])
            nc.sync.dma_start(dfbs[1:P, 0, :], dfbs[0:P - 1, ROWS_P, :])
            nc.scalar.dma_start(dfbs[0:P - 1, SLOTS - 1, :], dfbs[1:P, 1, :])

        for g in (0, 1):
            make_f16_copies(fbv[0], fbsv[0], 1.0, g)
        halo_dmas(fbv[0], fbsv[0])

        wv = w8b[:, :, :].rearrange("p s (r w) -> p s r w", w=W)
        w4v = w4f[:, :].rearrange("p (r w) -> p r w", w=W)
        out_v = out_d[:, :].rearrange("(p r) w -> p r w", r=ROWS_P)

        # ---- iterations ----
        for it in range(times):
            cfb, cfbs = fbv[it % 2], fbsv[it % 2]
            nfb, nfbs = fbv[(it + 1) % 2], fbsv[(it + 1) % 2]
            for g in (0, 1):
                r0 = 2 * g          # f32 row offset of this group
                s0 = 1 + r0         # fb slot offset
                a, b, c_, d = pr[g]

                def mul8(dst, k):
                    dr, dc = TAPS8[k]
                    wvg = wv[:, k, r0:r0 + 2, :]
                    if dc == 0:
                        src = cfbs[:, s0 + dr:s0 + 2 + dr, 0:W]
                    else:
                        src = cfb[:, s0 + dr:s0 + 2 + dr, 1 + dc:1 + dc + W]
                    nc.vector.tensor_mul(
                        dst[:, :].rearrange("p (r w) -> p r w", w=W), src, wvg)

                def add2(dst, x, y):
                    nc.vector.tensor_add(dst[:, :], x[:, :], y[:, :])

                mul8(a, 0); mul8(b, 1); add2(a, a, b)
                mul8(b, 2); mul8(c_, 3); add2(b, b, c_)
                add2(a, a, b)
                mul8(b, 4); mul8(c_, 5); add2(b, b, c_)
                mul8(c_, 6); mul8(d, 7); add2(c_, c_, d)
                add2(b, b, c_)
                add2(a, a, b)
                # center product fp32: c32 = w4f * f32
                c32 = pool.tile([P, 2 * W], F32, name=f"c32_{it}_{g}",
                                tag="c32", bufs=2)
                c32v = c32[:, :].rearrange("p (r w) -> p r w", w=W)
                nc.vector.tensor_mul(c32v, fs3[:, r0:r0 + 2, 1:1 + W],
                                     w4v[:, r0:r0 + 2, :])
                # final: f32 = tree * 2^it + c32 (in place, padded interior)
                nc.vector.scalar_tensor_tensor(
                    fs3[:, r0:r0 + 2, 1:1 + W],
                    a[:, :].rearrange("p (r w) -> p r w", w=W),
                    float(2.0 ** it), c32v, OP.mult, OP.add)
                if it != times - 1:
                    make_f16_copies(nfb, nfbs, float(2.0 ** -(it + 1)), g)
                else:
                    # overlap the store with the other group's compute
                    nc.sync.dma_start(out_v[:, r0:r0 + 2, :],
                                      fs3[:, r0:r0 + 2, 1:1 + W])
            if it != times - 1:
                halo_dmas(nfb, nfbs)

        if times == 0:
            nc.sync.dma_start(out_v, fs3[:, :, 1:1 + W])

    nc.compile()
    nc.m = get_hw_module(nc.m)
    return nc


_CACHE = {}


def _get(times: int):
    if times not in _CACHE:
        _CACHE[times] = _build(times)
    return _CACHE[times]


def kernel(affinity, feature, times, _trace=False, _trace_kwargs=None):
    t = int(times)
    nc = _get(t)
    aff = np.ascontiguousarray(affinity, dtype=np.float32)
    fea = np.ascontiguousarray(feature, dtype=np.float32)
    in_maps = [
        {"affinity": aff[b].reshape(CH, H * W), "feature": fea[b, 0]}
        for b in range(B)
    ]
    res = bass_utils.run_bass_kernel_spmd(
        nc, in_maps, core_ids=list(range(B)),
        trace=_trace, **(_trace_kwargs or {}),
    )
    out = np.stack([res.results[b]["out"] for b in range(B)])[:, None]
    if _trace:
        return out.astype(np.float32), res
    return out.astype(np.float32)



# revision 4
# speedup vs baseline: 1.0976x; 1.0976x over previous
"""AffinityPropagate prediction kernel for Trainium2 (8 NeuronCores).

Data-parallel over batch B=8: each core owns one image [480, 640].

Layout per core: 120 partitions x 4 image rows, rows padded to 642 cols.
All math runs in fp16 on the DVE at 2x mode. The center tap is folded
into the 9-tap product tree with all weights pre-halved (w' = w/2), so
the state evolves as f * 2^-iter and stays in fp16 range with no
per-iteration rescale; one final ACT copy un-scales by 2^times.

State is kept in two aligned fp16 forms (double-buffered): fbs (image
cols at row offset 0, serving dc==0 taps) and fb (cols at offset 1,
serving dc==+-1 taps); every window is then 4B-aligned so every
tensor_tensor runs at DVE 2x. Per iteration only 9 DVE instructions
run: 5 muls + 4 adds, with taps batched into multi-window access
patterns (tap-pair groups share one instruction) and the add tree
batched over concatenated product buffers. DVE writes fbs directly;
ACT re-creates fb (shifted copy); SBUF-SBUF DMAs refresh halo rows.
"""

import numpy as np
from contextlib import ExitStack

import concourse.bacc as bacc
import concourse.mybir as mybir
import concourse.tile as tile
from concourse import bass_utils
from concourse.ap import AP
from concourse.bass_interp import get_hw_module

B, CH, H, W = 8, 8, 480, 640
P = 120             # partitions (each holds ROWS_P rows)
ROWS_P = H // P     # 4
SLOTS = ROWS_P + 2  # + top/bottom halo row slots
WPAD = W + 2        # fb row: [0, img cols at 1..640, 0]
PX = ROWS_P * W     # 2560 compact px per partition

F32 = mybir.dt.float32
FP16 = mybir.dt.float16
I32 = mybir.dt.int32
AF = mybir.ActivationFunctionType
OP = mybir.AluOpType

# w9 slab order (taps grouped in instruction-pair order; center last):
#   0:(-1,-1) 1:(-1,+1) 2:(0,-1) 3:(0,+1) 4:(+1,-1) 5:(+1,+1)
#   6:(-1,0)  7:(+1,0)  8:center
# input affinity channel c (reference order) -> slab index:
SLAB_OF_CH = [0, 6, 1, 2, 3, 4, 7, 5]

# SCRATCH map (fp16 element offsets per partition)
AC16_O = 0            # [8, PX] fp16 copies of aff/2 in slab order (prep)
AST_O = 8 * PX        # 3-slot fp32 staging ring (2*PX units each)
ABS_O = 14 * PX       # 2-slot fp16 abs scratch ring (also sums32 after)
SUMS_O = 16 * PX
SUMA_O = 17 * PX
REC16_O = 18 * PX
REC32_O = 19 * PX     # fp32 (2*PX units); also fp32 out staging at tail
SCRR_O = 21 * PX      # fp32 recip scratch
SCR_UNITS = 23 * PX
RR_O = 0              # iter: two 3*PX-unit product rows
SCB = [6 * PX, 12 * PX]  # iter: X/D/E region, ring by parity

N_GPS = 0             # taps offloaded to gpsimd: 0=none, 1=center, 2=+D1


def _build(times: int):
    nc = bacc.Bacc("TRN2", debug=False, dynamic_dma_scratch_size=2048)
    aff_d = nc.dram_tensor("affinity", [CH, H * W], F32, kind="ExternalInput")
    feat_d = nc.dram_tensor("feature", [H, W], F32, kind="ExternalInput")
    out_d = nc.dram_tensor("out", [H, W], F32, kind="ExternalOutput")

    with tile.TileContext(nc) as tc, ExitStack() as ctx:
        pool = ctx.enter_context(tc.tile_pool(name="main", bufs=1))

        w9 = pool.tile([P, 9, PX], FP16)
        fbp = [pool.tile([P, SLOTS * WPAD], FP16, name=f"fb{i}")
               for i in range(2)]
        fbsp = [pool.tile([P, SLOTS * WPAD], FP16, name=f"fbs{i}")
                for i in range(2)]
        SCR = pool.tile([P, SCR_UNITS], FP16)

        scrf = SCR[:, :]

        def scr(o, n):
            return SCR[:, o:o + n]

        def scr32(o, n):
            return SCR[:, o:o + 2 * n].bitcast(F32)

        def mk(flat, off, dims):
            return AP(tensor=flat.tensor, offset=flat.offset + off,
                      ap=[list(flat.ap[0])] + [list(d) for d in dims])

        ac16 = scr(AC16_O, 8 * PX).rearrange("p (c x) -> p c x", x=PX)
        astage = [scr32(AST_O + 2 * PX * i, PX) for i in range(3)]
        abstmp = [scr(ABS_O + PX * i, PX) for i in range(2)]
        sums = scr(SUMS_O, PX)
        suma = scr(SUMA_O, PX)
        rec16 = scr(REC16_O, PX)
        sums32 = scr32(ABS_O, PX)
        rec32 = scr32(REC32_O, PX)
        scrr = scr32(SCRR_O, PX)

        def f3(t):
            return t[:, :].rearrange("p (s w) -> p s w", w=WPAD)

        fbv = [f3(t) for t in fbp]
        fbsv = [f3(t) for t in fbsp]

        # ---- zero-init padded state buffers (gpsimd; overlaps DMA) ----
        for t in fbp + fbsp:
            nc.gpsimd.memset(t[:, :], 0.0)

        # ---- feature load + initial fp16 state ----
        feat_v = feat_d[:, :].rearrange("(p r) w -> p r w", r=ROWS_P)
        nc.sync.dma_start(astage[0].rearrange("p (r w) -> p r w", w=W), feat_v)
        fst = astage[0].rearrange("p (r w) -> p r w", w=W)
        nc.scalar.activation(fbv[0][:, 1:5, 1:1 + W], fst, AF.Copy)
        nc.scalar.activation(fbsv[0][:, 1:5, 0:W], fst, AF.Copy)
        # initial halos
        nc.sync.dma_start(fbsv[0][1:P, 0, :], fbsv[0][0:P - 1, 4, :])
        nc.scalar.dma_start(fbsv[0][0:P - 1, 5, :], fbsv[0][1:P, 1, :])
        nc.scalar.dma_start(fbv[0][1:P, 0, :], fbv[0][0:P - 1, 4, :])
        nc.sync.dma_start(fbv[0][0:P - 1, 5, :], fbv[0][1:P, 1, :])

        # ---- affinity channels: load, fp16 copy (x0.5), accumulate sums ----
        aff_v = aff_d[:, :].rearrange("c (p x) -> c p x", x=PX)
        queues = [nc.sync, nc.scalar]
        for c in range(CH):
            st = astage[(c + 1) % 3]
            queues[c % 2].dma_start(st, aff_v[c])
            slab = SLAB_OF_CH[c]
            nc.scalar.activation(ac16[:, slab, :], st, AF.Copy, scale=0.5)
            if c == 0:
                nc.vector.tensor_scalar(
                    sums.bitcast(I32), ac16[:, slab, :].bitcast(I32),
                    0x7FFF7FFF, None, OP.bitwise_and)
                nc.vector.tensor_copy(suma, ac16[:, slab, :])
            else:
                ab = abstmp[c % 2]
                nc.vector.tensor_scalar(
                    ab.bitcast(I32), ac16[:, slab, :].bitcast(I32),
                    0x7FFF7FFF, None, OP.bitwise_and)
                nc.vector.tensor_add(sums, sums, ab)
                nc.vector.tensor_add(suma, suma, ac16[:, slab, :])

        # ---- normalize: w9[0:8] = (a/2) * (1/S); center = 0.5 - suma/S ----
        nc.vector.tensor_copy(sums32, sums)
        nc.vector.reciprocal_approx_accurate(rec32, sums32, scrr)
        nc.vector.tensor_scalar(rec16, rec32, 0.5, None, OP.mult)
        nc.vector.tensor_mul(w9[:, 0:8, :], ac16[:, :, :],
                             rec16.unsqueeze(1).broadcast_to([P, 8, PX]))
        nc.vector.tensor_mul(abstmp[0], suma, rec16)
        nc.vector.tensor_scalar(w9[:, 8, :], abstmp[0], -1.0, 0.5,
                                OP.mult, OP.add)

        w9f = w9[:, :, :].rearrange("p c x -> p (c x)")
        out_v = out_d[:, :].rearrange("(p r) w -> p r w", r=ROWS_P)

        # ---- iterations ----
        for it in range(times):
            cb, nb = it % 2, (it + 1) % 2
            cfb, cfbs = fbp[cb][:, :], fbsp[cb][:, :]
            nfb3, nfbs3 = fbv[nb], fbsv[nb]
            nfbs = fbsp[nb][:, :]
            last = it == times - 1
            sc = SCB[it % 2]
            M4 = [[PX, 2], [W, ROWS_P], [1, W]]

            # center product (interior only: ready right after prev L4)
            pe = (nc.gpsimd if N_GPS >= 1 else nc.vector)
            pe.tensor_mul(scr(sc + 5 * PX, PX),
                          mk(cfbs, WPAD, [[WPAD, ROWS_P], [1, W]]),
                          w9[:, 8, :].rearrange("p (s w) -> p s w", w=W))
            # vertical taps (-1,0),(+1,0) from fbs
            if N_GPS >= 2:
                nc.vector.tensor_mul(
                    scr(sc + 3 * PX, PX).rearrange("p (s w) -> p s w", w=W),
                    mk(cfbs, 0, [[WPAD, ROWS_P], [1, W]]),
                    w9[:, 6, :].rearrange("p (s w) -> p s w", w=W))
                nc.gpsimd.tensor_mul(
                    scr(sc + 4 * PX, PX).rearrange("p (s w) -> p s w", w=W),
                    mk(cfbs, 2 * WPAD, [[WPAD, ROWS_P], [1, W]]),
                    w9[:, 7, :].rearrange("p (s w) -> p s w", w=W))
            else:
                nc.vector.tensor_mul(
                    mk(scrf, sc + 3 * PX, M4[:1] + [[1, PX]]),
                    mk(cfbs, 0, [[2 * WPAD, 2], [WPAD, ROWS_P], [1, W]]),
                    mk(w9f, 6 * PX, M4))
            # row taps: A=(-1,*) B=(0,*) C=(+1,*) from fb, col offsets 0/2
            nc.vector.tensor_mul(mk(scrf, RR_O, M4),
                                 mk(cfb, 0, [[2, 2], [WPAD, ROWS_P], [1, W]]),
                                 mk(w9f, 0, M4))
            nc.vector.tensor_mul(mk(scrf, RR_O + 3 * PX, M4),
                                 mk(cfb, WPAD, [[2, 2], [WPAD, ROWS_P], [1, W]]),
                                 mk(w9f, 2 * PX, M4))
            nc.vector.tensor_mul(mk(scrf, RR_O + 2 * PX,
                                    [[3 * PX, 2], [W, ROWS_P], [1, W]]),
                                 mk(cfb, 2 * WPAD, [[2, 2], [WPAD, ROWS_P], [1, W]]),
                                 mk(w9f, 4 * PX, M4))
            # add tree over concatenated buffers
            nc.vector.tensor_add(scr(sc, 3 * PX), scr(RR_O, 3 * PX),
                                 scr(RR_O + 3 * PX, 3 * PX))
            l2 = mk(scrf, sc, [[2 * PX, 3], [1, PX]])
            nc.vector.tensor_tensor(l2, l2, mk(scrf, sc + PX,
                                               [[2 * PX, 3], [1, PX]]),
                                    OP.add)
            nc.vector.tensor_add(scr(sc, PX), scr(sc, PX),
                                 scr(sc + 2 * PX, PX))
            # final adds split {rows 0,3} then {rows 1,2} -> nfbs interior
            nc.vector.tensor_tensor(
                mk(nfbs, WPAD, [[3 * WPAD, 2], [1, W]]),
                mk(scrf, sc, [[3 * W, 2], [1, W]]),
                mk(scrf, sc + 4 * PX, [[3 * W, 2], [1, W]]), OP.add)
            if not last:
                nc.sync.dma_start(nfbs3[1:P, 0, :], nfbs3[0:P - 1, 4, :])
                nc.scalar.dma_start(nfbs3[0:P - 1, 5, :], nfbs3[1:P, 1, :])
            nc.vector.tensor_tensor(
                mk(nfbs, 2 * WPAD, [[WPAD, 2], [1, W]]),
                mk(scrf, sc + W, [[W, 2], [1, W]]),
                mk(scrf, sc + 4 * PX + W, [[W, 2], [1, W]]), OP.add)
            if not last:
                # rebuild fb (shifted copy) + its halos
                nc.scalar.activation(mk(fbp[nb][:, :], WPAD + 1,
                                        [[3 * WPAD, 2], [1, W]]),
                                     mk(nfbs, WPAD, [[3 * WPAD, 2], [1, W]]),
                                     AF.Copy)
                nc.scalar.dma_start(nfb3[1:P, 0, :], nfb3[0:P - 1, 4, :])
                nc.sync.dma_start(nfb3[0:P - 1, 5, :], nfb3[1:P, 1, :])
                nc.scalar.activation(mk(fbp[nb][:, :], 2 * WPAD + 1,
                                        [[WPAD, 2], [1, W]]),
                                     mk(nfbs, 2 * WPAD, [[WPAD, 2], [1, W]]),
                                     AF.Copy)
            else:
                # unscale to fp32 and store
                o32 = scr32(REC32_O, PX).rearrange("p (r w) -> p r w", w=W)
                sc16 = float(2.0 ** times)
                nc.scalar.activation(o32[:, 0:2, :], nfbs3[:, 1:3, 0:W],
                                     AF.Copy, scale=sc16)
                nc.sync.dma_start(out_v[:, 0:2, :], o32[:, 0:2, :])
                nc.scalar.activation(o32[:, 2:4, :], nfbs3[:, 3:5, 0:W],
                                     AF.Copy, scale=sc16)
                nc.scalar.dma_start(out_v[:, 2:4, :], o32[:, 2:4, :])

        if times == 0:
            nc.sync.dma_start(
                out_v, astage[0].rearrange("p (r w) -> p r w", w=W))

    nc.compile()
    nc.m = get_hw_module(nc.m)
    return nc


_CACHE = {}


def _get(times: int):
    if times not in _CACHE:
        _CACHE[times] = _build(times)
    return _CACHE[times]


def kernel(affinity, feature, times, _trace=False, _trace_kwargs=None):
    t = int(times)
    nc = _get(t)
    aff = np.ascontiguousarray(affinity, dtype=np.float32)
    fea = np.ascontiguousarray(feature, dtype=np.float32)
    in_maps = [
        {"affinity": aff[b].reshape(CH, H * W), "feature": fea[b, 0]}
        for b in range(B)
    ]
    res = bass_utils.run_bass_kernel_spmd(
        nc, in_maps, core_ids=list(range(B)),
        trace=_trace, **(_trace_kwargs or {}),
    )
    out = np.stack([res.results[b]["out"] for b in range(B)])[:, None]
    if _trace:
        return out.astype(np.float32), res
    return out.astype(np.float32)
